# revision 74
# baseline (speedup 1.0000x reference)
"""Trainium2 Bass kernel for nn_CausalContagionPredictor (gnn_message_passing).

Contract: kernel(**inputs) takes FULL unsharded numpy inputs (keys as in
setup_inputs()) and returns the full output (p_final[512], arr_final[512]).

Strategy (8 NeuronCores, row-sharded, cross-step software pipeline):
  - Core d owns source rows / nodes i in [64d, 64d+64).
  - Low-rank split of layer 1 (as before): static S in SBUF bf16, per-step
    per-pair bias column via a masked outer-product matmul
    ps_b = L.T @ (Mask * p_col), eliminating the p2 side-state.
  - The target axis is split into two chunks C0/C1 (host-side column
    permutation groups each core's first n0 nodes into C0). Each chunk gets
    its own sigmoid/scale/partition-max and its own ReduceScatter(max), so
    the ~2.2us/hop DMA+collective chain of one chunk overlaps compute of the
    other chunk and of the next step (next step's first pair-block depends
    only on chain0).
  - Per-step PE order: bias-mm(P0), P0 banks (mm2-C0 + mm3-C0), bias-mm(P1),
    P1 banks (mm2-C0 + mm3-C0), [tail0 -> chain0], all banks mm2-C1+mm3-C1,
    [tail1 -> chain1], optional dummy matmuls to hold the PE p-state.
  - State updates: half0 on DVE, half1 on Pool, so neither blocks the other
    half's next-step work.
  - Reference step 0 is peeled into host prep (only the <=4 shock rows
    have p>0 at step 0, so it is tiny in fp32 numpy); the device runs
    steps 1..9.
  - arr uses BIG=65536 in place of +inf on device.

Row->partition permutation: psum partition m holds local row i = m
(m in [0,32)) or i = m-32 (m in [64,96)); partitions 32-63 and 96-127 are
structurally-zero junk lanes (cgp rows there are 0).
"""

import numpy as np
import ml_dtypes

N = 512
D = 64
STEPS = 10
DEV_STEPS = 9                # device computes steps 1..9; step 0 is peeled
                             # into host prep (only shock rows have p>0)
N_CORES = 8
ROWS = N // N_CORES          # 64 source rows per core
PAIRS = ROWS // 2            # 32 even/odd row pairs
BIG = 65536.0

import os

N0 = int(os.environ.get("K_N0", "32"))   # nodes/core from chain0 (mult of 4)
C0 = N_CORES * N0            # columns in chunk 0
C1 = N - C0
Q0 = N0 // 2                 # pairs in P0
B0 = Q0 // 2                 # banks in P0 (bank t = pairs 2t, 2t+1)

DUMMY_MM = int(os.environ.get("K_DUMMY", "0"))
C1_EARLY = int(os.environ.get("K_C1E", "1"))   # C1-P0 banks run in X0
GSC_ENG = os.environ.get("K_GSC", "D")
DUMMY_MID = int(os.environ.get("K_DUMMYM", "0"))
MM3_LAG = int(os.environ.get("K_LAG", "3"))
LOOK = int(os.environ.get("K_LOOK", "4"))
R1ENG = os.environ.get("K_R1ENG", "D")
BANK_REORD = int(os.environ.get("K_BREORD", "0"))

# engine split (D=DVE, A=ACT, G=GPSIMD): relu1 per (pair, chunk),
# relu2 per (bank, chunk); PSUM-sourced relu2 only on D/A.
# Tuned per block: A-block (P0/C0) and B-block (P1/C0) feeds sized to PE;
# C1(P0) relu1s run in the chain0 overlap window (Pool/ACT-heavy).
R1C0_ENG = list(os.environ.get("K_R1C0", "DDDGDADG" * 4))
R1C1_ENG = list(os.environ.get("K_R1C1", "DDDGDADG" * 4))
R2C0_ENG = list(os.environ.get("K_R2C0", "ADADAADA" * 2))
R2C1_ENG = list(os.environ.get("K_R2C1", "ADADAADAADADADDD"))

_CACHE = {}


def _i_of_m(m):
    """psum partition m -> local row index i (or None for junk rows)."""
    if 0 <= m < 32:
        return m
    if 64 <= m < 96:
        return m - 32
    return None


def _row_ranges(a, b):
    """local rows [a,b) -> list of (partition_start, partition_stop)."""
    out = []
    if a < 32:
        out.append((a, min(b, 32)))
    if b > 32:
        out.append((64 + max(a, 32) - 32, 64 + b - 32))
    return out


def _col_perm():
    """compute-column c -> original target node (per the two-chunk layout)."""
    perm = np.empty(N, np.int64)
    n1 = ROWS - N0
    for c in range(C0):
        perm[c] = ROWS * (c // N0) + (c % N0)
    for c2 in range(C1):
        perm[C0 + c2] = ROWS * (c2 // n1) + N0 + (c2 % n1)
    return perm


def _build_bass(repeat=1, single_core=False, no_cc=False):
    import concourse.bacc as bacc
    import concourse.mybir as mybir
    import concourse.tile as tile
    import concourse.bass_isa as bass_isa

    fp32 = mybir.dt.float32
    bf16 = mybir.dt.bfloat16
    AF = mybir.ActivationFunctionType
    OP = mybir.AluOpType

    n_cores = 1 if single_core else N_CORES
    nc = bacc.Bacc("TRN2", target_bir_lowering=False, debug=False,
                   num_devices=n_cores)

    def dram_in(name, shape, dt):
        return nc.dram_tensor(name, shape, dt, kind="ExternalInput").ap()

    S_in = dram_in("S_in", [128, PAIRS * N], bf16)
    W2blk_in = dram_in("W2blk_in", [128, 64], bf16)
    LW3_in = dram_in("LW3_in", [128, 124], bf16)
    Ab1s_in = dram_in("Ab1s_in", [128, 32 * DEV_STEPS], fp32)
    L_in = dram_in("L_in", [128, 128], fp32)
    Mask_in = dram_in("Mask_in", [128, 32], fp32)
    cgp_in = dram_in("cgp_in", [128, N], fp32)
    b2bc_in = dram_in("b2bc_in", [128, 1], fp32)
    b3bc_in = dram_in("b3bc_in", [128, 1], fp32)
    pcol0_in = dram_in("pcol0_in", [128, 1], fp32)
    arr0_in = dram_in("arr0_in", [128, 1], fp32)

    p_out = nc.dram_tensor("p_out", [ROWS], fp32, kind="ExternalOutput").ap()
    arr_out = nc.dram_tensor("arr_out", [ROWS], fp32, kind="ExternalOutput").ap()

    half0_rr = _row_ranges(0, N0)      # partition ranges for chain0 rows
    half1_rr = _row_ranges(N0, ROWS)

    with tile.TileContext(nc) as tc:
        with tc.tile_pool(name="const", bufs=1) as cpool, \
             tc.tile_pool(name="h1", bufs=1) as h1pool, \
             tc.tile_pool(name="r2", bufs=8) as r2pool, \
             tc.tile_pool(name="tails", bufs=4) as tpool, \
             tc.tile_pool(name="ps_mm2", bufs=int(os.environ.get("K_PMM2","5")), space="PSUM") as pmm2, \
             tc.tile_pool(name="ps_h3", bufs=int(os.environ.get("K_PH3","1")), space="PSUM") as ph3, \
             tc.tile_pool(name="ps_bias", bufs=1, space="PSUM") as pbias, \
             tc.tile_pool(name="dram", bufs=2, space="DRAM") as dpool:

            # ---- load constants into SBUF (small ones first; S chunked so
            # step 0 starts after the first chunk) ----
            # S chunks lead the sync queue (step 0's relu1 gates on chunk
            # 0); all head constants go on the scalar queue, smallest first.
            p_colA = cpool.tile([128, 1], fp32, name="p_colA")
            nc.sync.dma_start(p_colA[:], pcol0_in[:])
            Mask = cpool.tile([128, 32], fp32, name="Mask")
            nc.sync.dma_start(Mask[:], Mask_in[:])
            S = cpool.tile([128, PAIRS * N], bf16, name="S")
            sl0 = slice(0, PAIRS * N // 8)
            nc.gpsimd.dma_start(S[:, sl0], S_in[:, sl0])
            for k in range(1, 8):
                sl = slice(k * PAIRS * N // 8, (k + 1) * PAIRS * N // 8)
                nc.sync.dma_start(S[:, sl], S_in[:, sl])
            Lmat = cpool.tile([128, 128], fp32, name="Lmat")
            nc.gpsimd.dma_start(Lmat[:], L_in[:])
            Ab1s = cpool.tile([128, 32 * DEV_STEPS], fp32, name="Ab1s")
            nc.gpsimd.dma_start(Ab1s[:], Ab1s_in[:])
            W2blk = cpool.tile([128, 64], bf16, name="W2blk")
            nc.scalar.dma_start(W2blk[:], W2blk_in[:])
            LW3 = cpool.tile([128, 124], bf16, name="LW3")
            nc.scalar.dma_start(LW3[:], LW3_in[:])
            b2bc = cpool.tile([128, 1], fp32, name="b2bc")
            nc.scalar.dma_start(b2bc[:], b2bc_in[:])
            b3bc = cpool.tile([128, 1], fp32, name="b3bc")
            nc.scalar.dma_start(b3bc[:], b3bc_in[:])
            arrA = cpool.tile([128, 1], fp32, name="arrA")
            nc.scalar.dma_start(arrA[:], arr0_in[:])
            cgp = cpool.tile([128, N], fp32, name="cgp")
            nc.scalar.dma_start(cgp[:], cgp_in[:])

            # persistent state / junk-safe tiles
            p_colB = cpool.tile([128, 1], fp32, name="p_colB")
            nc.vector.memset(p_colB[:], 0.0)
            arrB = cpool.tile([128, 1], fp32, name="arrB")
            nc.vector.memset(arrB[:], 0.0)
            cand_col = cpool.tile([128, 1], fp32, name="cand_col")
            nc.vector.memset(cand_col[:], 0.0)
            R = cpool.tile([128, 32], fp32, name="R")
            nc.vector.memset(R[:], 0.0)           # junk partitions stay 0

            p_cur, p_nxt = p_colA, p_colB
            arr_cur, arr_nxt = arrA, arrB

            for s_rep in range(DEV_STEPS * repeat):
                s = s_rep % DEV_STEPS

                def phase_head(pair_lo, pair_hi, rranges, tag, reng="D"):
                    """R build + bias matmul + biastile for a pair block."""
                    rns = nc.gpsimd if reng == "G" else nc.vector
                    for (a, b) in rranges:
                        # column range of R touched by these partitions
                        rns.tensor_scalar(
                            out=R[a:b, pair_lo:pair_hi],
                            in0=Mask[a:b, pair_lo:pair_hi],
                            scalar1=p_cur[a:b, 0:1], scalar2=None, op0=OP.mult)
                    kl = rranges[0][0]
                    kl = 64 if kl >= 64 else (32 if kl >= 32 else 0)
                    kh = rranges[-1][1]
                    ps_b = pbias.tile([128, 32], fp32, tag="psb")
                    nc.tensor.matmul(ps_b[:, pair_lo:pair_hi],
                                     Lmat[kl:kh, :], R[kl:kh, pair_lo:pair_hi],
                                     start=True, stop=True)
                    bt = tpool.tile([128, 32], fp32, tag=f"bt{tag}")
                    nc.vector.tensor_tensor(
                        bt[:, pair_lo:pair_hi], ps_b[:, pair_lo:pair_hi],
                        Ab1s[:, 32 * s + pair_lo:32 * s + pair_hi], OP.add)
                    return bt

                def relu1(i2, bt, clo, chi, eng):
                    """relu1 for pair i2 on target columns [clo, chi)."""
                    if h1s[i2] is None:
                        h1s[i2] = h1pool.tile([128, N], bf16, tag=f"h1_{i2}",
                                              name=f"h1_{i2}")
                    t = h1s[i2]
                    src_ap = S[:, i2 * N + clo:i2 * N + chi]
                    dst_ap = t[:, clo:chi]
                    bias_ap = bt[:, i2:i2 + 1]
                    if eng == "D":
                        nc.vector.tensor_scalar(
                            out=dst_ap, in0=src_ap, scalar1=bias_ap,
                            scalar2=0.0, op0=OP.add, op1=OP.max)
                    elif eng == "G":
                        nc.gpsimd.tensor_scalar(
                            out=dst_ap, in0=src_ap, scalar1=bias_ap,
                            scalar2=0.0, op0=OP.add, op1=OP.max)
                    else:
                        nc.scalar.activation(dst_ap, src_ap, AF.Relu,
                                             bias=bias_ap, scale=1.0)

                def relu2(ps, w, eng, tag):
                    r2 = r2pool.tile([128, w], bf16, tag=tag)
                    if eng == "D":
                        nc.vector.tensor_scalar(
                            out=r2[:], in0=ps[:], scalar1=b2bc[:, 0:1],
                            scalar2=0.0, op0=OP.add, op1=OP.max)
                    else:
                        nc.scalar.activation(r2[:], ps[:],
                                             AF.Relu, bias=b2bc[:, 0:1],
                                             scale=1.0)
                    return r2

                h1s = [None] * PAIRS

                # one PSUM bank holds h3 for both chunks (cols 0:C0 | C0:N)
                ps_h3 = ph3.tile([128, N], fp32, tag="psh3")

                def bank_mm2(t, clo, chi):
                    """mm2 pair for bank t on columns [clo, chi)."""
                    w = chi - clo
                    ps_2 = pmm2.tile([128, max(C0, C1)], fp32, tag="mm2")
                    nc.tensor.matmul(
                        ps_2[0:64, 0:w], W2blk[:], h1s[2 * t][:, clo:chi],
                        start=True, stop=True, tile_position=(0, 0))
                    nc.tensor.matmul(
                        ps_2[64:128, 0:w], W2blk[:],
                        h1s[2 * t + 1][:, clo:chi],
                        start=True, stop=True, tile_position=(0, 64))
                    return ps_2

                def bank_mm3(t, clo, chi, r2):
                    g = t % 8
                    lw = LW3[:, 60 - 4 * g:124 - 4 * g]
                    if t < 8:
                        nc.tensor.matmul(ps_h3[0:64, clo:chi], lw, r2[:],
                                         start=(t == 0), stop=(t == 7))
                    else:
                        nc.tensor.matmul(ps_h3[64:128, clo:chi], lw, r2[:],
                                         start=(t == 8), stop=(t == 15))

                def tail_chain(clo, chi, ccin_sz, ccout_sz, tag):
                    """sigmoid -> scale -> partition max -> store/RS/load."""
                    w = chi - clo
                    g = tpool.tile([128, w], fp32, tag=f"g{tag}")
                    nc.scalar.activation(g[:], ps_h3[:, clo:chi], AF.Sigmoid,
                                         bias=b3bc[:, 0:1], scale=1.0)
                    # gsc = (g * p_i) * cg  — fused, no pc tile needed
                    gsc = tpool.tile([128, w], fp32, tag=f"gsc{tag}")
                    ns = nc.gpsimd if GSC_ENG == "G" else nc.vector
                    ns.scalar_tensor_tensor(
                        out=gsc[:], in0=g[:], scalar=p_cur[:, 0:1],
                        in1=cgp[:, clo:chi], op0=OP.mult, op1=OP.mult)
                    par = tpool.tile([128, w], fp32, tag=f"par{tag}")
                    nc.gpsimd.partition_all_reduce(par[:], gsc[:], 128,
                                                   bass_isa.ReduceOp.max)
                    ccin = dpool.tile([ccin_sz], fp32, tag=f"ccin{tag}")
                    ccout = dpool.tile([ccout_sz], fp32, tag=f"ccout{tag}")
                    nc.sync.dma_start(ccin[:], par[0:1, :])
                    if single_core or no_cc:
                        nc.sync.dma_start(ccout[:], ccin[0:ccout_sz])
                    else:
                        nc.gpsimd.collective_compute(
                            "ReduceScatter", OP.max,
                            replica_groups=[list(range(N_CORES))],
                            ins=[ccin.opt()], outs=[ccout.opt()])
                    return ccout

                def load_cand(ccout, rranges):
                    off = 0
                    for (a, b) in rranges:
                        nc.sync.dma_start(cand_col[a:b, 0:1],
                                          ccout[off:off + (b - a)])
                        off += b - a

                def upd(rranges, eng, s=s):
                    # [*,1] state ops via tensor_scalar with per-partition
                    # scalar ptrs (Pool rejects TensorTensor in NCC)
                    ns = nc.vector if eng == "D" else nc.gpsimd
                    # p first: next step's R/bias wait only on this op
                    for (a, b) in rranges:
                        ns.tensor_scalar(
                            out=p_nxt[a:b, :], in0=cand_col[a:b, :],
                            scalar1=p_cur[a:b, 0:1], scalar2=None, op0=OP.max)
                    for (a, b) in rranges:
                        mask = tpool.tile([128, 1], fp32, tag=f"mask{a}")
                        ns.tensor_scalar(
                            out=mask[a:b, :], in0=cand_col[a:b, :],
                            scalar1=p_cur[a:b, 0:1], scalar2=None,
                            op0=OP.is_gt)
                        arrtmp = tpool.tile([128, 1], fp32, tag=f"arrt{a}")
                        ns.tensor_scalar(
                            out=arrtmp[a:b, :], in0=mask[a:b, :],
                            scalar1=float(s + 2) - BIG, scalar2=BIG,
                            op0=OP.mult, op1=OP.add)
                        ns.tensor_scalar(
                            out=arr_nxt[a:b, :], in0=arrtmp[a:b, :],
                            scalar1=arr_cur[a:b, 0:1], scalar2=None,
                            op0=OP.min)

                # ================= emission =================
                # Blocks: [head0 | C0-P0 banks | C1-P0 early banks]
                #         [head1 | C0-P1 banks -> tail0/chain0]
                #         [C1 remaining banks -> tail1/chain1]
                # 2-pair relu1 lookahead, 2-bank mm3 lag (carried across
                # blocks of the same chunk).
                bts = [None, None]
                emitted_r1 = set()

                def r1(i2, chunk):
                    if i2 >= PAIRS or (i2, chunk) in emitted_r1:
                        return
                    bt = bts[0] if i2 < Q0 else bts[1]
                    if bt is None:
                        return
                    emitted_r1.add((i2, chunk))
                    if chunk == 0:
                        relu1(i2, bt, 0, C0, R1C0_ENG[i2])
                    else:
                        relu1(i2, bt, C0, N, R1C1_ENG[i2])

                # per-chunk mm3 pipeline state
                pend = {0: [], 1: []}

                def run_banks(banks, chunk, pat, rtag, look):
                    clo, chi = (0, C0) if chunk == 0 else (C0, N)
                    w = chi - clo
                    for t in banks:
                        r1(2 * t, chunk)
                        r1(2 * t + 1, chunk)
                        ps_2 = bank_mm2(t, clo, chi)
                        # lookahead relu1s queue BEFORE relu2 so feed engines
                        # don't stall in-order on the PSUM dependency
                        for la in range(2 * t + 2, 2 * t + 2 + look):
                            r1(la, chunk)
                        r2 = relu2(ps_2[:, 0:w], w, pat[t], rtag)
                        pend[chunk].append((t, r2))
                        while len(pend[chunk]) > MM3_LAG:
                            tt, rr = pend[chunk].pop(0)
                            bank_mm3(tt, clo, chi, rr)

                def flush(chunk):
                    clo, chi = (0, C0) if chunk == 0 else (C0, N)
                    while pend[chunk]:
                        tt, rr = pend[chunk].pop(0)
                        bank_mm3(tt, clo, chi, rr)

                def border(banks, chunk):
                    """start/stop banks pinned; G-fed banks pushed late."""
                    banks = list(banks)
                    if len(banks) < 3 or not BANK_REORD:
                        return banks
                    pat = R1C0_ENG if chunk == 0 else R1C1_ENG
                    head, tail = banks[0], banks[-1]
                    mid = banks[1:-1]
                    fast = [t for t in mid if pat[2 * t] != "G"
                            and pat[2 * t + 1] != "G"]
                    slow = [t for t in mid if t not in fast]
                    return [head] + fast + slow + [tail]

                bts[0] = phase_head(0, Q0, half0_rr, "0")
                r1(0, 0)
                r1(1, 0)
                run_banks(border(range(0, B0), 0), 0, R2C0_ENG, "r2c0", LOOK)
                flush(0)
                r1(0, 1)
                r1(1, 1)
                run_banks(range(0, C1_EARLY), 1, R2C1_ENG, "r2c1", LOOK)
                # PE keep-warm while waiting for chain1 of previous step
                if DUMMY_MID:
                    ps_d = pbias.tile([128, 256], fp32, tag="dum")
                    for _ in range(DUMMY_MID):
                        nc.tensor.matmul(ps_d[0:64, :], W2blk[:],
                                         S[:, 0:256], start=True, stop=True)

                bts[1] = phase_head(Q0, PAIRS, half1_rr, "1", reng=R1ENG)
                r1(2 * B0, 0)
                r1(2 * B0 + 1, 0)
                run_banks(border(range(B0, 16), 0), 0, R2C0_ENG, "r2c0", LOOK)
                flush(0)
                ccout0 = tail_chain(0, C0, C0, N0, "0")
                load_cand(ccout0, half0_rr)

                # remaining C1 banks (overlap chain0)
                run_banks(border(range(C1_EARLY, 8), 1), 1, R2C1_ENG, "r2c1", LOOK)
                run_banks(border(range(8, 16), 1), 1, R2C1_ENG, "r2c1", LOOK)
                flush(1)
                ccout1 = tail_chain(C0, N, C1, ROWS - N0, "1")
                load_cand(ccout1, half1_rr)
                # PE keep-warm dummies (no consumers, read-only constants)
                if DUMMY_MM:
                    ps_d = pbias.tile([128, 256], fp32, tag="dum")
                    for _ in range(DUMMY_MM):
                        nc.tensor.matmul(ps_d[0:64, :], W2blk[:],
                                         S[:, 0:256], start=True, stop=True)
                # state updates (emitted last so DVE/Pool queues aren't
                # blocked ahead of C1 work; half0 on DVE, half1 on Pool)
                upd(half0_rr, "D")
                upd(half1_rr, "D" if s == DEV_STEPS - 1 else "G")
                p_cur, p_nxt = p_nxt, p_cur
                arr_cur, arr_nxt = arr_nxt, arr_cur

            # ---- outputs (two queues so issues overlap) ----
            nc.sync.dma_start(p_out[0:32], p_cur[0:32, 0:1])
            nc.scalar.dma_start(p_out[32:64], p_cur[64:96, 0:1])
            nc.sync.dma_start(arr_out[0:32], arr_cur[0:32, 0:1])
            nc.scalar.dma_start(arr_out[32:64], arr_cur[64:96, 0:1])

    nc.compile()
    return nc


def _host_prep(inputs):
    """Build per-core input maps (numpy)."""
    bf = ml_dtypes.bfloat16
    cg = np.asarray(inputs["causal_graph"], np.float32)
    nf = np.asarray(inputs["node_features"], np.float32)
    shock = np.asarray(inputs["shock_nodes"]).astype(np.int64)
    W1 = np.asarray(inputs["W1"], np.float32)
    b1 = np.asarray(inputs["b1"], np.float32)
    W2 = np.asarray(inputs["W2"], np.float32)
    b2 = np.asarray(inputs["b2"], np.float32)
    W3 = np.asarray(inputs["W3"], np.float32)
    b3 = float(np.asarray(inputs["b3"], np.float32)[0])

    A = nf @ W1[:D]                      # [N, D]
    B = nf @ W1[D:2 * D]                 # [N, D]
    w_cg, w_p, w_s, w_f = W1[2 * D], W1[2 * D + 1], W1[2 * D + 2], W1[2 * D + 3]
    f0d = np.abs(nf[:, 0][:, None] - nf[None, :, 0])     # [N, N]

    perm = _col_perm()                   # compute col -> node
    Bp = B[perm]                         # [N, D] permuted targets
    f0dp = f0d[:, perm]
    cgperm = cg[:, perm]

    p0 = np.zeros(N, np.float32)
    arr0 = np.full(N, BIG, np.float32)
    p0[shock] = 1.0
    arr0[shock] = 0.0

    # ---- peel reference step 0 on host (only shock rows have p>0) ----
    rows0 = np.where(p0 > 0)[0]
    nfr = nf[rows0]                                   # [R, 64]
    Rn = len(rows0)
    comb = np.concatenate([
        np.broadcast_to(nfr[:, None, :], (Rn, N, D)),
        np.broadcast_to(nf[None, :, :], (Rn, N, D)),
        np.stack([cg[rows0],
                  np.ones((Rn, N), np.float32),
                  np.zeros((Rn, N), np.float32),
                  f0d[rows0]], axis=-1)], axis=-1).astype(np.float32)
    h = np.maximum(comb @ W1 + b1, 0.0)
    h = np.maximum(h @ W2 + b2, 0.0)
    t0 = 1.0 / (1.0 + np.exp(-(h @ np.asarray(inputs["W3"], np.float32)
                               + np.asarray(inputs["b3"], np.float32))))[..., 0]
    newp = np.where(cg[rows0] > 0, t0 * cg[rows0], 0.0)
    cand0 = newp.max(axis=0) if Rn else np.zeros(N, np.float32)
    improved0 = cand0 > p0
    p0 = np.maximum(p0, cand0).astype(np.float32)
    arr0 = np.where(improved0, np.minimum(arr0, 1.0), arr0).astype(np.float32)

    W2blk = np.zeros((128, 64), np.float32)              # block-diag W2
    W2blk[0:64, 0:32] = W2
    W2blk[64:128, 32:64] = W2
    W2blk = W2blk.astype(bf)
    LW3 = np.zeros((128, 124), np.float32)
    for r in range(4):
        LW3[32 * r:32 * (r + 1), 60 + r] = W3[:, 0]
    LW3 = LW3.astype(bf)
    b2bc = np.tile(b2, 4).reshape(128, 1).astype(np.float32)

    # L matrix for bias outer product: ps_b[m, c] = w_p[d(m)] * p[row]
    Lmat = np.zeros((128, 128), np.float32)
    for k in range(128):
        i = _i_of_m(k)
        if i is None:
            continue
        for m in range(128):
            if (i % 2) == (1 if m >= 64 else 0):
                Lmat[k, m] = w_p[m % 64]
    Maskm = np.zeros((128, 32), np.float32)
    for k in range(128):
        i = _i_of_m(k)
        if i is not None:
            Maskm[k, i // 2] = 1.0

    in_maps = []
    for d in range(N_CORES):
        rows = slice(ROWS * d, ROWS * (d + 1))
        cg_d = cgperm[rows]              # [64, 512] permuted cols
        f0_d = f0dp[rows]
        A_d = A[rows]                    # [64, 64]

        # S_pack [128, PAIRS*N] bf16 (columns permuted)
        S_pack = np.empty((128, PAIRS * N), np.float32)
        BT = Bp.T                        # [D, N] permuted
        for i2 in range(PAIRS):
            ie, io = 2 * i2, 2 * i2 + 1
            blk = slice(i2 * N, (i2 + 1) * N)
            S_pack[0:64, blk] = BT + np.outer(w_cg, cg_d[ie]) + np.outer(w_f, f0_d[ie])
            S_pack[64:128, blk] = BT + np.outer(w_cg, cg_d[io]) + np.outer(w_f, f0_d[io])
        S_pack = S_pack.astype(bf)

        # Ab1s [128, 32*DEV_STEPS] fp32: device step s = reference step s+1
        Ab1s = np.empty((128, 32 * DEV_STEPS), np.float32)
        for s in range(DEV_STEPS):
            base = b1[None, :] + (np.float32(s + 1) / np.float32(STEPS)) * w_s[None, :]
            blk = slice(32 * s, 32 * (s + 1))
            Ab1s[0:64, blk] = (A_d[0::2] + base).T      # [64h, 32i2]
            Ab1s[64:128, blk] = (A_d[1::2] + base).T

        # cgp [128, N]: partition m -> cg_d[i(m)] or 0
        cgp = np.zeros((128, N), np.float32)
        for m in range(128):
            i = _i_of_m(m)
            if i is not None:
                cgp[m] = cg_d[i]

        pcol0 = np.zeros((128, 1), np.float32)
        arr0c = np.zeros((128, 1), np.float32)
        for m in range(128):
            i = _i_of_m(m)
            if i is not None:
                pcol0[m, 0] = p0[ROWS * d + i]
                arr0c[m, 0] = arr0[ROWS * d + i]

        in_maps.append({
            "S_in": S_pack, "W2blk_in": W2blk, "LW3_in": LW3,
            "Ab1s_in": Ab1s, "L_in": Lmat, "Mask_in": Maskm,
            "cgp_in": cgp, "b2bc_in": b2bc,
            "b3bc_in": np.full((128, 1), b3, np.float32),
            "pcol0_in": pcol0, "arr0_in": arr0c,
        })
    return in_maps, b3


def kernel(**inputs):
    from concourse.bass_utils import run_bass_kernel_spmd

    in_maps, _b3 = _host_prep(inputs)
    if "nc" not in _CACHE:
        _CACHE["nc"] = _build_bass()
    nc = _CACHE["nc"]

    res = run_bass_kernel_spmd(nc, in_maps, core_ids=list(range(N_CORES)))
    p_full = np.empty(N, np.float32)
    arr_full = np.empty(N, np.float32)
    for d in range(N_CORES):
        p_full[ROWS * d:ROWS * (d + 1)] = res.results[d]["p_out"]
        arr_full[ROWS * d:ROWS * (d + 1)] = res.results[d]["arr_out"]
    arr_full = np.where(arr_full >= BIG / 2, np.inf, arr_full).astype(np.float32)
    return p_full, arr_full
